# revision 14
# baseline (speedup 1.0000x reference)
"""CSWin transformer block on 8 trn2 NeuronCores, data-parallel over B=8.

Per-core layout plan (one image, L=4096 tokens, C=192 channels):
  LN1 token-major -> PE-transpose to imgT [97, 2, L] (row 96 = ones, feeds
  bias rows of host-augmented weights).  qkv^T via PE with HEAD-PADDED
  channel layout [128, 2, L] (head h at partitions 32h..32h+24) so the
  S^T = K@Q^T matmuls can row-tile 4 heads concurrently (tile_position).
  v token-major, gathered into window order, 32-interleaved [v(24)|ones(8)]
  so the AV matmul's ones columns produce softmax sums in the pad rows.
  Softmax: exp on ACT psum->sbuf (bf16), sums via ones-cols, reciprocal,
  PE broadcast matmul (E_bc), one DVE tensor_tensor per window; pad rows
  normalize to exactly 1.0 which feeds proj bias via a host proj_w row.
  LeFF: LN2 -> transpose (reuses imgT) -> lin1+gelu -> depthwise 3x3 as
  9 PSUM-accumulated diagonal matmuls (+bias+gelu) -> lin2 (+bias via
  K=1 ones-row matmul) -> residual.  LeFF hidden runs in two L-halves
  to fit SBUF; activations bf16, contractions fp32 in PSUM.
"""

import numpy as np
import ml_dtypes

B, H, W, C = 8, 64, 64, 192
L = H * W
HALF = C // 2
NH = 4
D = HALF // NH          # 24
SPLIT = 8
NWIN = 8
SCALE = D ** -0.5
HID = 4 * C             # 768
NCHUNK = L // 128       # 32
EPS = 1e-5
BF16 = ml_dtypes.bfloat16

_CACHE = {}


def _make_tc_cls():
    import concourse.mybir as mybir
    import concourse.tile as tile

    uid = [0]

    def split_multi_waits(nc):
        for fn in nc.m.functions:
            for bb in fn.blocks:
                new_il, changed = [], False
                for inst in bb.instructions:
                    si = inst.sync_info
                    if si is not None and len(si.on_wait) > 1:
                        waits = list(si.on_wait)
                        for w in waits[:-1]:
                            uid[0] += 1
                            nop = mybir.InstNoOp(name=f"I-wfix-{uid[0]}",
                                                 ins=[], outs=[])
                            nop.engine = inst.engine
                            nop.sync_info = mybir.SyncInfo(on_wait=[w],
                                                           on_update=[])
                            new_il.append(nop)
                        inst.sync_info = mybir.SyncInfo(
                            on_wait=[waits[-1]], on_update=list(si.on_update))
                        changed = True
                    new_il.append(inst)
                if changed:
                    bb.instructions = new_il

    class TCFixed(tile.TileContext):
        def __exit__(self, et, ev, tb):
            r = super().__exit__(et, ev, tb)
            if et is None:
                split_multi_waits(self.nc)
            return r

    return TCFixed


def _prep_consts(qkv_w, proj_w, proj_b, n1g, n1b, n2g, n2b,
                 lin1_w, lin1_b, dw_w, dw_b, lin2_w, lin2_b):
    f32 = np.float32
    qkv_eff = (n1g[:, None] * qkv_w).astype(f32)
    bias1 = (n1b @ qkv_w).astype(f32)                 # [3C]

    def pad_qk(wsl, bsl, scale):
        # wsl [192, 96] -> [97, 2, 128] head-padded (kc-major second dim)
        out = np.zeros((97, 2, 128), f32)
        for h in range(NH):
            cs = slice(32 * h, 32 * h + D)
            out[:96, 0, cs] = wsl[:96, D * h:D * h + D] * scale
            out[:96, 1, cs] = wsl[96:, D * h:D * h + D] * scale
            out[96, 0, cs] = bsl[D * h:D * h + D] * scale
        return out

    consts = {}
    for b in range(2):
        qs = slice(HALF * b, HALF * b + HALF)
        ks = slice(C + HALF * b, C + HALF * b + HALF)
        consts[f"qT_w{b}"] = pad_qk(qkv_eff[:, qs], bias1[qs], SCALE)
        consts[f"kT_w{b}"] = pad_qk(qkv_eff[:, ks], bias1[ks], 1.0)

    v_w = np.zeros((97, 2, 2, 96), f32)               # [p, branch, kc, N]
    for b in range(2):
        vs = slice(2 * C + HALF * b, 2 * C + HALF * b + HALF)
        v_w[:96, b, 0] = qkv_eff[:96, vs]
        v_w[96, b, 0] = bias1[vs]
        v_w[:96, b, 1] = qkv_eff[96:, vs]
    consts["v_w"] = v_w

    proj_p = np.zeros((128, 2, C), f32)               # [p(k-pad), branch, C]
    for b in range(2):
        for h in range(NH):
            proj_p[32 * h:32 * h + D, b] = \
                proj_w[HALF * b + D * h: HALF * b + D * h + D]
    proj_p[24, 0] += proj_b                           # pad row == 1.0 post-norm
    consts["proj_p"] = proj_p.astype(BF16)

    e_bc = np.zeros((128, 128), f32)
    for h in range(NH):
        e_bc[32 * h + 24, 32 * h:32 * (h + 1)] = 1.0
    consts["e_bc"] = e_bc
    consts["ident"] = np.eye(128, dtype=f32)

    lin1_eff = (n2g[:, None] * lin1_w).astype(f32)
    lin1_bias = (lin1_b + n2b @ lin1_w).astype(f32)
    lin1_aug = np.zeros((97, 2, HID), f32)
    lin1_aug[:96, 0] = lin1_eff[:96]
    lin1_aug[96, 0] = lin1_bias
    lin1_aug[:96, 1] = lin1_eff[96:]
    consts["lin1_aug"] = lin1_aug

    dwd = np.zeros((128, 54, 128), f32)               # [p, cc*9+tap, M]
    for cc in range(6):
        for dy in range(3):
            for dx in range(3):
                np.fill_diagonal(
                    dwd[:, cc * 9 + dy * 3 + dx, :],
                    dw_w[128 * cc:128 * cc + 128, 0, dy, dx])
    consts["dwd"] = dwd.astype(BF16)
    consts["lin2w"] = np.ascontiguousarray(
        lin2_w.reshape(6, 128, C).transpose(1, 0, 2)).astype(BF16)
    consts["dwb"] = np.ascontiguousarray(
        dw_b.reshape(6, 128).T).astype(f32)           # [128, 6]
    consts["lin2b"] = lin2_b.reshape(1, C).astype(f32)
    return consts


def _build(consts):
    import concourse.bass as bass
    import concourse.mybir as mybir
    dt = mybir.dt
    TC = _make_tc_cls()
    nc = bass.Bass()
    x_d = nc.declare_dram_parameter("x", [L, C], dt.float32, isOutput=False)
    o_d = nc.declare_dram_parameter("o", [L, C], dt.float32, isOutput=True)
    cst = {}
    for name, arr in consts.items():
        dtt = dt.bfloat16 if arr.dtype == BF16 else dt.float32
        cst[name] = nc.declare_dram_parameter(name, list(arr.shape), dtt,
                                              isOutput=False)
    with TC(nc) as tc:
        _emit(nc, tc, mybir, x_d, o_d, cst)
    return nc


def _win_ap(t, b, w, part_sl, j4=None):
    """Contiguous window slice.  Branch-0 planes of q/k/attT-era tiles
    are stored WINDOW-MAJOR (col = 512*w + 8*h + j); branch-1 windows
    are naturally l-contiguous.  Either way window (b,w) = cols
    [512w, 512w+512) and m-chunk j4 = 128 cols within it."""
    c0 = 512 * w + (128 * j4 if j4 is not None else 0)
    c1 = c0 + (128 if j4 is not None else 512)
    return t[part_sl, b, c0:c1]


def _emit(nc, tc, mybir, x_d, o_d, cst):
    dt = mybir.dt
    AF = mybir.ActivationFunctionType
    ALU = mybir.AluOpType
    AX = mybir.AxisListType
    f32, bf = dt.float32, dt.bfloat16

    import contextlib
    est = contextlib.ExitStack()
    with est:
        main = est.enter_context(tc.tile_pool(name="main", bufs=1))
        xt = main.tile([128, NCHUNK, C], f32)      # x; reused as out buffer
        imgT = main.tile([128, 2, L], f32)         # imgT; reused as y2T
        yt = main.tile([128, NCHUNK, C], f32)
        w_qT = [main.tile([97, 2, 128], f32, name=f"wq{b}") for b in range(2)]
        w_kT = [main.tile([97, 2, 128], f32, name=f"wk{b}") for b in range(2)]
        w_v = main.tile([97, 2, 2, 96], f32)
        w_vb = main.tile([97, 2, 2, 96], bf)
        w_proj = main.tile([128, 2, C], bf)
        identt = main.tile([128, 128], f32)
        e_bct = main.tile([128, 128], f32)
        onesrow = main.tile([1, 128], f32)
        epst = main.tile([128, 1], f32)
        nc.vector.memset(epst[:], EPS)

        nc.sync.dma_start(out=xt[:],
                          in_=x_d[:].rearrange("(n p) c -> p n c", p=128))
        for b in range(2):
            nc.sync.dma_start(out=w_qT[b][:], in_=cst[f"qT_w{b}"][:])
            nc.sync.dma_start(out=w_kT[b][:], in_=cst[f"kT_w{b}"][:])
        nc.sync.dma_start(out=w_v[:], in_=cst["v_w"][:])
        nc.vector.tensor_copy(out=w_vb[:], in_=w_v[:])
        nc.sync.dma_start(out=w_proj[:], in_=cst["proj_p"][:])
        nc.sync.dma_start(out=identt[:], in_=cst["ident"][:])
        nc.sync.dma_start(out=e_bct[:], in_=cst["e_bc"][:])
        nc.vector.memset(onesrow[:], 1.0)
        nc.vector.memset(imgT[96:97, 0, :], 1.0)

        # ---------------- LN (shared emitter) ----------------
        def emit_ln_transpose(src, dstT, lnp, lnps):
            for j in range(NCHUNK):
                st6 = lnp.tile([128, 6], f32, tag="st6")
                mv = lnp.tile([128, 2], f32, tag="mv")
                nc.vector.bn_stats(out=st6[:], in_=src[:, j])
                nc.vector.bn_aggr(out=mv[:], in_=st6[:])
                nc.scalar.activation(out=mv[:, 1:2], in_=mv[:, 1:2],
                                     func=AF.Sqrt, bias=epst[:], scale=1.0)
                nc.vector.reciprocal(out=mv[:, 1:2], in_=mv[:, 1:2])
                img = lnp.tile([128, C], f32, tag="img")
                nc.vector.tensor_scalar(
                    out=img[:], in0=src[:, j], scalar1=mv[:, 0:1],
                    scalar2=mv[:, 1:2], op0=ALU.subtract, op1=ALU.mult)
                for kc in range(2):
                    pst = lnps.tile([96, 128], f32, tag="tp")
                    nc.tensor.transpose(pst[:], img[:, 96 * kc:96 * kc + 96],
                                        identt[:])
                    nc.vector.tensor_copy(
                        out=dstT[0:96, kc, 128 * j:128 * (j + 1)], in_=pst[:])

        with tc.tile_pool(name="ln1", bufs=3) as lnp, \
             tc.tile_pool(name="ln1ps", bufs=4, space="PSUM") as lnps:
            emit_ln_transpose(xt, imgT, lnp, lnps)

        attn_scope = tc.tile_pool(name="attn", bufs=1)
        attn = attn_scope.__enter__()
        qT = attn.tile([128, 2, L], bf)
        kT = attn.tile([128, 2, L], bf)
        v_aug = attn.tile([128, 2, NWIN, 4, 128], bf)
        attT = attn.tile([128, 2, L], bf)
        nc.vector.memset(v_aug[:], 0.0)
        for h in range(NH):
            nc.vector.memset(v_aug[:, :, :, :, 32 * h + 24:32 * h + 32], 1.0)

        # ---------------- qkv^T (head-padded) + v (window-gathered) -------
        # imgTw: branch-0 window-major copy of imgT (col = 512w + 8h + j).
        with tc.tile_pool(name="imgtw", bufs=1) as twp, \
             tc.tile_pool(name="qkvps", bufs=4, space="PSUM") as qps:
            imgTw = twp.tile([97, 2, L], bf)
            for kc in range(2):
                nc.vector.tensor_copy(
                    out=imgTw[:, kc, :].rearrange(
                        "p (w h j) -> p h w j", w=SPLIT, h=64, j=SPLIT),
                    in_=imgT[0:97, kc, :].rearrange(
                        "p (h w j) -> p h w j", h=64, w=SPLIT, j=SPLIT))
            for b in range(2):
                for wt, dstT in ((w_qT[b], qT), (w_kT[b], kT)):
                    for nt in range(8):
                        ps = qps.tile([128, 512], f32, tag="qk")
                        for kc in range(2):
                            kk = 97 if kc == 0 else 96
                            nc.tensor.matmul(
                                ps[:], wt[:kk, kc, :],
                                imgT[:kk, kc, 512 * nt:512 * (nt + 1)],
                                start=(kc == 0), stop=(kc == 1))
                        if b == 0:
                            # scatter l-order cols to window-major positions
                            nc.scalar.copy(
                                out=dstT[:, 0, :].rearrange(
                                    "p (w h j) -> p h w j",
                                    w=SPLIT, h=64, j=SPLIT)[
                                    :, 8 * nt:8 * nt + 8],
                                in_=ps[:].rearrange(
                                    "p (h w j) -> p h w j",
                                    h=SPLIT, w=SPLIT, j=SPLIT))
                        else:
                            nc.scalar.copy(
                                out=dstT[:, 1, 512 * nt:512 * (nt + 1)],
                                in_=ps[:])
                for w in range(NWIN):
                    for j4 in range(4):
                        ps = qps.tile([128, 96], f32, tag="v")
                        for kc in range(2):
                            kk = 97 if kc == 0 else 96
                            if b == 0:
                                lhsT = imgTw[:kk, kc,
                                             512 * w + 128 * j4:
                                             512 * w + 128 * (j4 + 1)]
                                rhs = w_vb[:kk, b, kc, :]
                            else:
                                l0 = 512 * w + 128 * j4
                                lhsT = imgT[:kk, kc, l0:l0 + 128]
                                rhs = w_v[:kk, b, kc, :]
                            nc.tensor.matmul(ps[:], lhsT, rhs,
                                             start=(kc == 0), stop=(kc == 1))
                        nc.vector.tensor_copy(
                            out=v_aug[:, b, w, j4, :].rearrange(
                                "p (h e) -> p h e", e=32)[:, :, 0:D],
                            in_=ps[:].rearrange("p (h e) -> p h e", e=D))

        # ---------------- attention ----------------
        with tc.tile_pool(name="att_sb", bufs=2) as asb, \
             tc.tile_pool(name="att_exp", bufs=1) as aexp, \
             tc.tile_pool(name="st_ps", bufs=1, space="PSUM") as stps, \
             tc.tile_pool(name="av_ps", bufs=2, space="PSUM") as avps, \
             tc.tile_pool(name="bc_ps", bufs=2, space="PSUM") as bcps:
            for b in range(2):
                for w in range(NWIN):
                    expT = aexp.tile([128, 4, 4, 512], bf, tag="expT")
                    for j4 in range(4):
                        stp = stps.tile([128, 2048], f32, tag="st")
                        for h in range(NH):
                            psl = slice(32 * h, 32 * h + 32)
                            nc.tensor.matmul(
                                stp[:, 512 * h:512 * (h + 1)],
                                _win_ap(kT, b, w, psl, j4),
                                _win_ap(qT, b, w, psl),
                                start=True, stop=True,
                                tile_position=(32 * h, 0))
                        nc.scalar.activation(
                            out=expT[:, j4],
                            in_=stp[:].rearrange("p (h n) -> p h n", h=4),
                            func=AF.Exp)
                    avp = avps.tile([128, 512], f32, tag="av")
                    for j4 in range(4):
                        for h in range(NH):
                            nc.tensor.matmul(
                                avp[32 * h:32 * h + 32, :],
                                v_aug[:, b, w, j4, 32 * h:32 * h + 32],
                                expT[:, j4, h],
                                start=(j4 == 0 and h == 0),
                                stop=(j4 == 3 and h == 3),
                                tile_position=(0, 32 * h))
                    rec = asb.tile([128, 512], f32, tag="rec")
                    nc.vector.reciprocal(out=rec[:], in_=avp[:])
                    bcp = bcps.tile([128, 512], f32, tag="bc")
                    nc.tensor.matmul(bcp[:], e_bct[:], rec[:],
                                     start=True, stop=True)
                    bcs = asb.tile([128, 512], f32, tag="bcs")
                    nc.scalar.copy(out=bcs[:], in_=bcp[:])
                    if b == 0:
                        # scatter window-order n back to l-order columns
                        dst = attT[:, 0, :].rearrange(
                            "p (h w j) -> p h w j", h=64, w=SPLIT,
                            j=SPLIT)[:, :, w, :]
                        nc.vector.tensor_tensor(
                            out=dst,
                            in0=avp[:].rearrange("p (h j) -> p h j", j=SPLIT),
                            in1=bcs[:].rearrange("p (h j) -> p h j", j=SPLIT),
                            op=ALU.mult)
                    else:
                        nc.vector.tensor_tensor(
                            out=attT[:, 1, 512 * w:512 * (w + 1)],
                            in0=avp[:], in1=bcs[:], op=ALU.mult)

        # ---------------- proj + residual ----------------
        with tc.tile_pool(name="prps", bufs=4, space="PSUM") as prps:
            for j in range(NCHUNK):
                ps = prps.tile([128, C], f32, tag="pr")
                for b in range(2):
                    nc.tensor.matmul(ps[:], attT[:, b, 128 * j:128 * (j + 1)],
                                     w_proj[:, b], start=(b == 0),
                                     stop=(b == 1))
                nc.vector.scalar_tensor_tensor(
                    out=yt[:, j], in0=ps[:], scalar=1.0, in1=xt[:, j],
                    op0=ALU.mult, op1=ALU.add)
        attn_scope.__exit__(None, None, None)

        # ---------------- LN2 + transpose (imgT reused as y2T) ------------
        with tc.tile_pool(name="ln2", bufs=3) as lnp2, \
             tc.tile_pool(name="ln2ps", bufs=4, space="PSUM") as lnps2:
            emit_ln_transpose(yt, imgT, lnp2, lnps2)

        # ---------------- LeFF in two L-halves ----------------
        # tT holds 34 input h-rows per half, 66-wide padded rows:
        # stored row s (= global h - hin0), data col w at flat 66*s + w + 2;
        # flat cols 66*s+1 and 66*s+67 are zero pads, so the depthwise conv
        # runs as 9 FLAT-shifted diagonal matmuls (w-edge wraps hit pads).
        TLEN = 34 * 66 + 2
        leff = est.enter_context(tc.tile_pool(name="leff", bufs=1))
        tT = leff.tile([128, 6, TLEN], bf)
        t2T = leff.tile([128, 6, 2048], bf)
        w_lin1 = leff.tile([97, 2, HID], f32)
        w_dwd = leff.tile([128, 54, 128], bf)
        w_lin2 = leff.tile([128, 6, C], bf)
        dwbt = leff.tile([128, 6], f32)
        lin2bt = leff.tile([1, C], f32)
        nc.sync.dma_start(out=w_lin1[:], in_=cst["lin1_aug"][:])
        nc.sync.dma_start(out=w_dwd[:], in_=cst["dwd"][:])
        nc.sync.dma_start(out=w_lin2[:], in_=cst["lin2w"][:])
        nc.sync.dma_start(out=dwbt[:], in_=cst["dwb"][:])
        nc.sync.dma_start(out=lin2bt[:], in_=cst["lin2b"][:])
        nc.vector.memset(tT[:], 0.0)

        for half in range(2):
            hin0 = 0 if half == 0 else 30      # first stored input h-row
            hout0 = 32 * half
            # lin1 + gelu into padded tT rows (input rows hin0..hin0+34)
            with tc.tile_pool(name=f"l1ps{half}", bufs=4, space="PSUM") as l1p:
                row_tiles = [(0, 8), (8, 8), (16, 8), (24, 8), (32, 2)]
                for mc in range(6):
                    for (s0, nr) in row_tiles:
                        ps = l1p.tile([128, 512], f32, tag="l1")
                        l0 = (hin0 + s0) * 64
                        nlen = nr * 64
                        for kc in range(2):
                            kk = 97 if kc == 0 else 96
                            nc.tensor.matmul(
                                ps[:, :nlen],
                                w_lin1[:kk, kc, 128 * mc:128 * (mc + 1)],
                                imgT[:kk, kc, l0:l0 + nlen],
                                start=(kc == 0), stop=(kc == 1))
                        dst = tT[:, mc, 66 * s0 + 2:
                                 66 * s0 + 2 + 66 * nr].rearrange(
                            "p (r w) -> p r w", w=66)[:, :, 0:64]
                        nc.scalar.activation(
                            out=dst,
                            in_=ps[:, :nlen].rearrange(
                                "p (r w) -> p r w", w=64),
                            func=AF.Gelu)
            # depthwise 3x3 + bias + gelu: flat-shifted diag matmuls
            with tc.tile_pool(name=f"dwps{half}", bufs=4, space="PSUM") as dwp:
                blocks = [(0, 6), (6, 6), (12, 6), (18, 6), (24, 6), (30, 2)]
                taps = [(1, 1), (0, 0), (0, 1), (0, 2), (1, 0),
                        (1, 2), (2, 0), (2, 1), (2, 2)]
                for cc in range(6):
                    for (boff, bn) in blocks:
                        g0 = hout0 + boff          # global out h of block
                        g1 = g0 + bn
                        ps = dwp.tile([128, 66 * 6], f32, tag="dw")
                        for ti, (dy, dx) in enumerate(taps):
                            rlo = max(g0, 1 - dy)
                            rhi = min(g1, 65 - dy)
                            src0 = 66 * (rlo + dy - 1 - hin0) + dx
                            ln = 66 * (rhi - rlo)
                            nc.tensor.matmul(
                                ps[:, 66 * (rlo - g0):66 * (rlo - g0) + ln],
                                w_dwd[:, cc * 9 + dy * 3 + dx, :],
                                tT[:, cc, src0:src0 + ln],
                                start=(ti == 0), stop=(ti == 8))
                        nc.scalar.activation(
                            out=t2T[:, cc, 64 * boff:64 * boff + 64 * bn
                                    ].rearrange("p (r w) -> p r w", w=64),
                            in_=ps[:, :66 * bn].rearrange(
                                "p (r w) -> p r w", w=66)[:, :, 1:65],
                            func=AF.Gelu, bias=dwbt[:, cc:cc + 1], scale=1.0)
            # lin2 + bias + residual -> out (xt reused)
            with tc.tile_pool(name=f"l2ps{half}", bufs=4, space="PSUM") as l2p:
                for jj in range(16):
                    j = 16 * half + jj
                    ps = l2p.tile([128, C], f32, tag="l2")
                    for cc in range(6):
                        nc.tensor.matmul(
                            ps[:], t2T[:, cc, 128 * jj:128 * (jj + 1)],
                            w_lin2[:, cc], start=(cc == 0), stop=False)
                    nc.tensor.matmul(ps[:], onesrow[:], lin2bt[:],
                                     start=False, stop=True)
                    nc.vector.scalar_tensor_tensor(
                        out=xt[:, j], in0=ps[:], scalar=1.0, in1=yt[:, j],
                        op0=ALU.mult, op1=ALU.add)
        nc.sync.dma_start(out=o_d[:].rearrange("(n p) c -> p n c", p=128),
                          in_=xt[:])
    est.close()


def _get_program(inputs):
    key = "prog"
    if key in _CACHE:
        return _CACHE[key]
    consts = _prep_consts(
        inputs["qkv_w"], inputs["proj_w"], inputs["proj_b"],
        inputs["norm1_g"], inputs["norm1_b"], inputs["norm2_g"],
        inputs["norm2_b"], inputs["lin1_w"], inputs["lin1_b"],
        inputs["dw_w"], inputs["dw_b"], inputs["lin2_w"], inputs["lin2_b"])
    nc = _build(consts)
    _CACHE[key] = (nc, consts)
    return nc, consts


def kernel(**inputs):
    from concourse.bass_utils import run_bass_kernel_spmd
    np_in = {k: np.asarray(v) for k, v in inputs.items()}
    nc, consts = _get_program(np_in)
    x = np_in["x"].astype(np.float32)
    in_maps = []
    for core in range(8):
        m = {"x": np.ascontiguousarray(x[core])}
        m.update(consts)
        in_maps.append(m)
    res = run_bass_kernel_spmd(nc, in_maps, list(range(8)))
    out = np.stack([res.results[i]["o"] for i in range(8)], axis=0)
    return out.astype(np.float32)
